# revision 18
# baseline (speedup 1.0000x reference)
"""Trainium2 Bass kernel for nn_InterpretableBottleneck (vq_codebook).

Data-parallel over batch: B=256 is sharded 32-per-core across 8 NeuronCores.
Codebook and all weights are replicated. A single small AllReduce combines the
code-usage histogram (for perplexity) and the squared-error sum (for vq_loss).

Numerical note: the reference computes
    dists = ||m||^2 + ||c||^2 - 2 m @ c.T          (float32)
where ||m||^2 ~ 256 dominates, so dists are quantized to ~1.5e-5 granularity.
The argmin winner therefore depends on that exact f32 rounding structure, and
replicating it makes the winner robust to tiny numeric differences. The kernel
computes neg_dists = fl(2*C) - fl(msq + csq) elementwise (exactly mirroring the
reference rounding, negated) and takes argmax with first-index tie-break, which
matches jnp.argmin's tie-break.
"""

import sys

for _p in ("/opt/trn_rl_repo", "/root/.axon_site/_ro/trn_rl_repo"):
    if _p not in sys.path:
        sys.path.insert(0, _p)

from contextlib import ExitStack

import numpy as np

import concourse.bacc as bacc
import concourse.bass as bass
import concourse.mybir as mybir
import concourse.tile as tile
from concourse import bass_isa
from concourse.bass import IndirectOffsetOnAxis
from concourse.bass_utils import run_bass_kernel_spmd
from concourse.masks import make_identity

F32 = mybir.dt.float32
I16 = mybir.dt.int16
U32 = mybir.dt.uint32
AF = mybir.ActivationFunctionType
ALU = mybir.AluOpType
AX = mybir.AxisListType

N_CORES = 8
B = 32          # batch per core
S = 128         # sequence
D = 1024        # per-modality model dim
D3 = 3 * D      # 3072
H2 = 512
H = 256
K = 8192        # codebook entries
NS = 1000       # speakers
POL, PROS, VIS = 3, 16, 10
LN_EPS = 1e-5
CC = 0.25

_CACHE = {}
LAST_RESULTS = None


def _bcast_ap(src, parts, drop_first=False):
    """AP that reads src's (first) row replicated across `parts` rows."""
    rest = list(src.ap[1:]) if drop_first else list(src.ap)
    return bass.AP(tensor=src.tensor, offset=src.offset,
                   ap=[[0, parts]] + rest)


def _build():
    nc = bacc.Bacc("TRN2", target_bir_lowering=False, debug=False,
                   num_devices=N_CORES)

    u = {m: nc.dram_tensor(f"u_{m}", [B, S, D], F32, kind="ExternalInput").ap()
         for m in ("t", "a", "v")}
    ext_in = {}
    for nm, shp in (("W1", [D3, H2]), ("b1", [H2]), ("ln1_g", [H2]),
                    ("ln1_b", [H2]), ("W2", [H2, H]), ("b2", [H]),
                    ("ln2_g", [H]), ("ln2_b", [H]), ("codebook", [K, H]),
                    ("dW1", [H, 128]), ("db1", [128]), ("dW2", [128, NS]),
                    ("db2", [NS]), ("pW", [H, POL]), ("pb", [POL]),
                    ("prW", [H, PROS]), ("prb", [PROS]), ("vW", [H, VIS]),
                    ("vb", [VIS])):
        ext_in[nm] = nc.dram_tensor(nm, shp, F32, kind="ExternalInput").ap()

    outs = {}
    for nm, shp in (("M_out", [B, H]), ("mraw_out", [B, H]),
                    ("spk_out", [B, NS]), ("pol_out", [B, POL]),
                    ("pros_out", [B, PROS]), ("vis_out", [B, VIS]),
                    ("scal_out", [1, 2])):
        outs[nm] = nc.dram_tensor(nm, shp, F32, kind="ExternalOutput").ap()

    with tile.TileContext(nc) as tc:
        with ExitStack() as ctx:
            _body(ctx, tc, u, ext_in, outs)

    nc.compile()
    return nc


def _layernorm(nc, pool, x_sb, out_sb, g_bc, b_bc, n, eps_ap):
    """LayerNorm rows of x_sb [B, n] -> out_sb, matching the reference."""
    stats = pool.tile([B, 6], F32, tag="ln_stats", name="ln_stats")
    mv = pool.tile([B, 2], F32, tag="ln_mv", name="ln_mv")
    nc.vector.bn_stats(out=stats, in_=x_sb)
    nc.vector.bn_aggr(out=mv, in_=stats)
    std = pool.tile([B, 1], F32, tag="ln_std", name="ln_std")
    nc.scalar.activation(out=std, in_=mv[:, 1:2], func=AF.Sqrt, bias=eps_ap)
    rstd = pool.tile([B, 1], F32, tag="ln_rstd", name="ln_rstd")
    nc.vector.reciprocal(out=rstd, in_=std)
    nmr = pool.tile([B, 1], F32, tag="ln_nmr", name="ln_nmr")
    nc.vector.scalar_tensor_tensor(out=nmr, in0=mv[:, 0:1], scalar=-1.0,
                                   in1=rstd, op0=ALU.mult, op1=ALU.mult)
    xc = pool.tile([B, n], F32, tag="ln_xc", name="ln_xc", bufs=1)
    nc.scalar.activation(out=xc, in_=x_sb, func=AF.Identity, bias=nmr,
                         scale=rstd)
    nc.vector.tensor_tensor(out=xc, in0=xc, in1=g_bc, op=ALU.mult)
    nc.vector.tensor_tensor(out=out_sb, in0=xc, in1=b_bc, op=ALU.add)


def _body(ctx, tc, u, ein, eout):
    nc = tc.nc
    MODS = ("t", "a", "v")

    singles = ctx.enter_context(tc.tile_pool(name="singles", bufs=1))
    upool = ctx.enter_context(tc.tile_pool(name="upool", bufs=3))
    cbpool = ctx.enter_context(tc.tile_pool(name="cbpool", bufs=4))
    wpool = ctx.enter_context(tc.tile_pool(name="wpool", bufs=3))
    sqpool = ctx.enter_context(tc.tile_pool(name="sqpool", bufs=2))
    small = ctx.enter_context(tc.tile_pool(name="small", bufs=2))
    bigp = ctx.enter_context(tc.tile_pool(name="bigp", bufs=1))
    psT = ctx.enter_context(tc.tile_pool(name="psT", bufs=2, space="PSUM"))
    psMM = ctx.enter_context(tc.tile_pool(name="psMM", bufs=2, space="PSUM"))
    psC = ctx.enter_context(tc.tile_pool(name="psC", bufs=3, space="PSUM"))
    dram = ctx.enter_context(tc.tile_pool(name="dram", bufs=1, space="DRAM"))

    def sg(shape, dtype, name):
        return singles.tile(shape, dtype, name=name, tag=name)

    def sm(shape, dtype, name, bufs=None):
        return small.tile(shape, dtype, name=name, tag=name, bufs=bufs)

    # ---------------- constants / weights in SBUF ----------------
    ident = sg([128, 128], F32, "ident")
    make_identity(nc, ident)
    ones_1xB = sg([1, B], F32, "ones_1xB")
    nc.vector.memset(ones_1xB, 1.0)
    ones_Bx1 = sg([B, 1], F32, "ones_Bx1")
    nc.vector.memset(ones_Bx1, 1.0)
    ones_128x1 = sg([128, 1], F32, "ones_128x1")
    nc.vector.memset(ones_128x1, 1.0)
    iota_k = sg([B, K], I16, "iota_k")
    nc.gpsimd.iota(out=iota_k, pattern=[[1, K]], base=0, channel_multiplier=0)
    eps_ap = sg([B, 1], F32, "eps_ap")
    nc.vector.memset(eps_ap, LN_EPS)
    e10_ap = sg([128, 1], F32, "e10_ap")
    nc.vector.memset(e10_ap, 1e-10)

    def bload(nm, parts, n):
        t = sg([parts, n], F32, f"{nm}_sb")
        nc.sync.dma_start(out=t, in_=_bcast_ap(ein[nm], parts))
        return t

    b1_sb = bload("b1", 1, H2)
    b2_sb = bload("b2", 1, H)
    db1_sb = bload("db1", 1, 128)
    db2_sb = bload("db2", 1, NS)
    pb_sb = bload("pb", 1, POL)
    prb_sb = bload("prb", 1, PROS)
    vb_sb = bload("vb", 1, VIS)
    g1_bc = bload("ln1_g", B, H2)
    bb1_bc = bload("ln1_b", B, H2)
    g2_bc = bload("ln2_g", B, H)
    bb2_bc = bload("ln2_b", B, H)

    # =====================================================================
    # 1) Mean-pool over S per modality.
    # Tile: partition p = q*32 + b (q = d//256), free = [s_in(8), d_q(256)]
    # -> every DMA descriptor is a contiguous 1KB quarter of a d-row.
    # =====================================================================
    accs = {}
    for m in MODS:
        acc = sg([128, 256], F32, f"acc_{m}")
        accs[m] = acc
        for si in range(16):
            ut = upool.tile([128, 8, 256], F32, tag="u_tile", name="u_tile")
            for q in range(4):
                nc.sync.dma_start(
                    out=ut[q * 32:(q + 1) * 32, :, :],
                    in_=u[m][:, si * 8:(si + 1) * 8,
                             q * 256:(q + 1) * 256])
            if si == 0:
                nc.vector.tensor_reduce(
                    out=acc, in_=ut.rearrange("p s d -> p d s"),
                    axis=AX.X, op=ALU.add)
            else:
                part = upool.tile([128, 256], F32, tag="u_part", name="u_part")
                nc.vector.tensor_reduce(
                    out=part, in_=ut.rearrange("p s d -> p d s"),
                    axis=AX.X, op=ALU.add)
                nc.vector.tensor_tensor(out=acc, in0=acc, in1=part, op=ALU.add)

    # xT chunks (transpose + 1/S scaling): k-chunk (m, q, c2)
    xT = []
    for mi, m in enumerate(MODS):
        for q in range(4):
            for c2 in range(2):
                ps = psT.tile([128, 128], F32, tag="psT", name="ps_xT")
                nc.tensor.transpose(ps[:, 0:B],
                                    accs[m][q * 32:(q + 1) * 32,
                                            c2 * 128:(c2 + 1) * 128],
                                    ident[q * 32:q * 32 + B,
                                          q * 32:q * 32 + B],
                                    tile_position=(q * 32, 0))
                t = sg([128, B], F32, f"xT_{mi}_{q}_{c2}")
                nc.scalar.activation(out=t, in_=ps[:, 0:B], func=AF.Copy,
                                     scale=1.0 / S)
                xT.append((t, mi * D + q * 256 + c2 * 128))

    # =====================================================================
    # 2) Compressor -> m_raw
    # =====================================================================
    ps_h = psMM.tile([B, H2], F32, tag="psMM", name="ps_h")
    for i, (t, off) in enumerate(xT):
        w = wpool.tile([128, H2], F32, tag="w1", name="w1")
        nc.sync.dma_start(out=w, in_=ein["W1"][off:off + 128, :])
        nc.tensor.matmul(ps_h, t, w, start=(i == 0), stop=False)
    nc.tensor.matmul(ps_h, ones_1xB, b1_sb, start=False, stop=True)
    h_sb = sm([B, H2], F32, "h_sb", bufs=1)
    nc.scalar.activation(out=h_sb, in_=ps_h, func=AF.Relu)
    hn = sm([B, H2], F32, "hn", bufs=1)
    _layernorm(nc, small, h_sb, hn, g1_bc, bb1_bc, H2, eps_ap)

    hnT = []
    for c in range(4):
        ps = psT.tile([128, 128], F32, tag="psT", name="ps_hnT")
        nc.tensor.transpose(ps[:, 0:B], hn[:, c * 128:(c + 1) * 128],
                            ident[0:B, 0:B])
        t = sm([128, B], F32, f"hnT{c}", bufs=1)
        nc.scalar.copy(out=t, in_=ps[:, 0:B])
        hnT.append(t)

    ps_m = psMM.tile([B, H], F32, tag="psMM", name="ps_m")
    for c in range(4):
        w = wpool.tile([128, H], F32, tag="w2", name="w2")
        nc.sync.dma_start(out=w, in_=ein["W2"][c * 128:(c + 1) * 128, :])
        nc.tensor.matmul(ps_m, hnT[c], w, start=(c == 0), stop=False)
    nc.tensor.matmul(ps_m, ones_1xB, b2_sb, start=False, stop=True)
    m_pre = sm([B, H], F32, "m_pre", bufs=1)
    nc.scalar.copy(out=m_pre, in_=ps_m)
    m_raw = sg([B, H], F32, "m_raw")
    _layernorm(nc, small, m_pre, m_raw, g2_bc, bb2_bc, H, eps_ap)
    nc.sync.dma_start(out=eout["mraw_out"], in_=m_raw)

    mT = []
    for c in range(2):
        ps = psT.tile([128, 128], F32, tag="psT", name="ps_mT")
        nc.tensor.transpose(ps[:, 0:B], m_raw[:, c * 128:(c + 1) * 128],
                            ident[0:B, 0:B])
        t = sg([128, B], F32, f"mT{c}")
        nc.scalar.copy(out=t, in_=ps[:, 0:B])
        mT.append(t)
    msq = sg([B, 1], F32, "msq")
    msq_scr = sm([B, H], F32, "msq_scr", bufs=1)
    nc.vector.scalar_tensor_tensor(out=msq_scr, in0=m_raw, scalar=1.0,
                                   in1=m_raw, op0=ALU.bypass, op1=ALU.mult,
                                   accum_out=msq)

    # =====================================================================
    # 3) Codebook: transpose to cT (2 chunks of 128 d-rows), csq into nd.
    # =====================================================================
    cT = [sg([128, K], F32, f"cT{c}") for c in range(2)]
    for kt in range(64):
        cbt = cbpool.tile([128, H], F32, tag="cb_tile", name="cb_tile")
        nc.sync.dma_start(out=cbt, in_=ein["codebook"][kt * 128:(kt + 1) * 128, :])
        for c in range(2):
            ps = psT.tile([128, 128], F32, tag="psT", name="ps_cT")
            nc.tensor.transpose(ps, cbt[:, c * 128:(c + 1) * 128], ident)
            nc.scalar.copy(out=cT[c][:, kt * 128:(kt + 1) * 128], in_=ps)

    # nd row 0 accumulates csq; broadcast to the other B-1 rows; then
    # T1 = fl(msq+csq) in place; then neg_dists = fl(2C) - T1 in place.
    nd = bigp.tile([B, K], F32, tag="big", name="nd")
    for n in range(16):
        sl = bass.ts(n, 512)
        ps = psC.tile([B, 512], F32, tag="psC", name="ps_csq")
        for c in range(2):
            sq = sqpool.tile([128, 512], F32, tag="ct_sq", name="ct_sq")
            nc.scalar.activation(out=sq, in_=cT[c][:, sl], func=AF.Square)
            nc.tensor.matmul(ps[0:1, :], ones_128x1, sq,
                             start=(c == 0), stop=(c == 1))
        nc.scalar.copy(out=nd[0:1, sl], in_=ps[0:1, :])
    csq_dram = dram.tile([1, K], F32, name="csq_dram")
    nc.sync.dma_start(out=csq_dram, in_=nd[0:1, :])
    nc.sync.dma_start(out=nd[1:B, :], in_=_bcast_ap(csq_dram, B - 1,
                                                    drop_first=True))
    nc.vector.tensor_scalar(out=nd, in0=nd, scalar1=msq, scalar2=None,
                            op0=ALU.add)

    for n in range(16):
        sl = bass.ts(n, 512)
        ps = psC.tile([B, 512], F32, tag="psC", name="ps_C")
        nc.tensor.matmul(ps, mT[0], cT[0][:, sl], start=True, stop=False)
        nc.tensor.matmul(ps, mT[1], cT[1][:, sl], start=False, stop=True)
        nc.vector.scalar_tensor_tensor(out=nd[:, sl], in0=ps, scalar=2.0,
                                       in1=nd[:, sl], op0=ALU.mult,
                                       op1=ALU.subtract)

    mx8 = sg([B, 8], F32, "mx8")
    nc.vector.max(out=mx8, in_=nd)
    idx8 = sg([B, 8], U32, "idx8")
    nc.vector.max_index(out=idx8, in_max=mx8, in_values=nd)

    q_sb = sg([B, H], F32, "q_sb")
    nc.gpsimd.indirect_dma_start(
        out=q_sb, out_offset=None, in_=ein["codebook"],
        in_offset=IndirectOffsetOnAxis(ap=idx8[:, 0:1], axis=0))
    nc.sync.dma_start(out=eout["M_out"], in_=q_sb)

    # =====================================================================
    # 4) Histogram + sumsq partials, AllReduce, scalar outputs.
    # =====================================================================
    idx_f = sm([B, 1], F32, "idx_f", bufs=1)
    nc.vector.tensor_copy(out=idx_f, in_=idx8[:, 0:1])
    onehot = bigp.tile([B, K], F32, tag="big", name="onehot")
    nc.vector.tensor_scalar(out=onehot, in0=iota_k, scalar1=idx_f,
                            scalar2=None, op0=ALU.is_equal)
    cc_in = dram.tile([1, 8200], F32, name="cc_in")
    cc_out = dram.tile([1, 8200], F32, name="cc_out")
    for n in range(16):
        sl = bass.ts(n, 512)
        ps = psC.tile([B, 512], F32, tag="psC", name="ps_cnt")
        nc.tensor.matmul(ps[0:1, :], ones_Bx1, onehot[:, sl],
                         start=True, stop=True)
        cnt_st = sm([1, 512], F32, "cnt_st")
        nc.scalar.copy(out=cnt_st, in_=ps[0:1, :])
        nc.sync.dma_start(out=cc_in[0:1, sl], in_=cnt_st)

    diff = sm([B, H], F32, "diff", bufs=1)
    nc.vector.tensor_tensor(out=diff, in0=q_sb, in1=m_raw, op=ALU.subtract)
    d2 = sm([B, H], F32, "d2", bufs=1)
    ss = sm([B, 1], F32, "ss", bufs=1)
    nc.vector.scalar_tensor_tensor(out=d2, in0=diff, scalar=1.0, in1=diff,
                                   op0=ALU.bypass, op1=ALU.mult, accum_out=ss)
    ssr = sm([B, 1], F32, "ssr", bufs=1)
    nc.gpsimd.partition_all_reduce(out_ap=ssr, in_ap=ss, channels=B,
                                   reduce_op=bass_isa.ReduceOp.add)
    zpad = sm([1, 8], F32, "zpad", bufs=1)
    nc.vector.memset(zpad, 0.0)
    nc.sync.dma_start(out=cc_in[0:1, 8192:8193], in_=ssr[0:1, 0:1])
    nc.sync.dma_start(out=cc_in[0:1, 8193:8200], in_=zpad[0:1, 0:7])

    nc.gpsimd.collective_compute(
        "AllReduce", ALU.add,
        replica_groups=[list(range(N_CORES))],
        ins=[cc_in.opt()], outs=[cc_out.opt()])

    cnt2 = sm([128, 64], F32, "cnt2", bufs=1)
    nc.sync.dma_start(out=cnt2,
                      in_=cc_out[0:1, 0:8192].rearrange("o (p f) -> (o p) f",
                                                        p=128))
    lnv = sm([128, 64], F32, "lnv", bufs=1)
    nc.scalar.activation(out=lnv, in_=cnt2, func=AF.Ln, scale=1.0 / 256,
                         bias=e10_ap)
    ent_scr = sm([128, 64], F32, "ent_scr", bufs=1)
    entp = sm([128, 1], F32, "entp", bufs=1)
    nc.vector.scalar_tensor_tensor(out=ent_scr, in0=lnv, scalar=1.0 / 256,
                                   in1=cnt2, op0=ALU.mult, op1=ALU.mult,
                                   accum_out=entp)
    entr = sm([128, 1], F32, "entr", bufs=1)
    nc.gpsimd.partition_all_reduce(out_ap=entr, in_ap=entp, channels=128,
                                   reduce_op=bass_isa.ReduceOp.add)
    scal_sb = sg([1, 2], F32, "scal_sb")
    et = sm([1, 1], F32, "et", bufs=1)
    nc.scalar.activation(out=et, in_=entr[0:1, 0:1], func=AF.Exp, scale=-1.0)
    nc.scalar.mul(out=scal_sb[0:1, 1:2], in_=et, mul=1.0 / K)
    sst = sm([1, 1], F32, "sst", bufs=1)
    nc.sync.dma_start(out=sst, in_=cc_out[0:1, 8192:8193])
    nc.scalar.mul(out=scal_sb[0:1, 0:1], in_=sst,
                  mul=(1.0 + CC) / (256.0 * H))
    nc.sync.dma_start(out=eout["scal_out"], in_=scal_sb)

    # =====================================================================
    # 5) Heads on M (= q_sb); psum results DMA'd straight to DRAM outputs.
    # =====================================================================
    qT = []
    for c in range(2):
        ps = psT.tile([128, 128], F32, tag="psT", name="ps_qT")
        nc.tensor.transpose(ps[:, 0:B], q_sb[:, c * 128:(c + 1) * 128],
                            ident[0:B, 0:B])
        t = sm([128, B], F32, f"qT{c}", bufs=1)
        nc.scalar.copy(out=t, in_=ps[:, 0:B])
        qT.append(t)

    dw1_sb = sm([128, 128, 2], F32, "dw1_sb", bufs=1)
    nc.sync.dma_start(out=dw1_sb, in_=ein["dW1"].rearrange("(c p) n -> p n c",
                                                           c=2))
    ps1 = psMM.tile([B, 128], F32, tag="psMM", name="ps_spk1")
    nc.tensor.matmul(ps1, qT[0], dw1_sb[:, :, 0], start=True, stop=False)
    nc.tensor.matmul(ps1, qT[1], dw1_sb[:, :, 1], start=False, stop=False)
    nc.tensor.matmul(ps1, ones_1xB, db1_sb, start=False, stop=True)
    h1 = sm([B, 128], F32, "h1", bufs=1)
    nc.scalar.activation(out=h1, in_=ps1, func=AF.Relu)
    ps_h1T = psT.tile([128, 128], F32, tag="psT", name="ps_h1T")
    nc.tensor.transpose(ps_h1T[:, 0:B], h1, ident[0:B, 0:B])
    h1T = sm([128, B], F32, "h1T", bufs=1)
    nc.scalar.copy(out=h1T, in_=ps_h1T[:, 0:B])

    dw2_sb = sm([128, NS], F32, "dw2_sb", bufs=1)
    nc.sync.dma_start(out=dw2_sb, in_=ein["dW2"])
    for n0, n1 in ((0, 512), (512, NS)):
        ps = psMM.tile([B, 512], F32, tag="psMM", name="ps_spk2")
        nc.tensor.matmul(ps[:, 0:n1 - n0], h1T, dw2_sb[:, n0:n1],
                         start=True, stop=False)
        nc.tensor.matmul(ps[:, 0:n1 - n0], ones_1xB, db2_sb[0:1, n0:n1],
                         start=False, stop=True)
        spk_st = sm([B, 512], F32, "spk_st")
        nc.scalar.copy(out=spk_st[:, 0:n1 - n0], in_=ps[:, 0:n1 - n0])
        nc.sync.dma_start(out=eout["spk_out"][:, n0:n1],
                          in_=spk_st[:, 0:n1 - n0])

    for wnm, bap, ncols, onm in (("pW", pb_sb, POL, "pol_out"),
                                 ("prW", prb_sb, PROS, "pros_out"),
                                 ("vW", vb_sb, VIS, "vis_out")):
        w = sm([128, ncols, 2], F32, f"w_{wnm}", bufs=1)
        nc.sync.dma_start(out=w, in_=ein[wnm].rearrange("(c p) n -> p n c",
                                                        c=2))
        ps = psMM.tile([B, 512], F32, tag="psMM", name=f"ps_{wnm}")
        nc.tensor.matmul(ps[:, 0:ncols], qT[0], w[:, :, 0],
                         start=True, stop=False)
        nc.tensor.matmul(ps[:, 0:ncols], qT[1], w[:, :, 1],
                         start=False, stop=False)
        nc.tensor.matmul(ps[:, 0:ncols], ones_1xB, bap,
                         start=False, stop=True)
        ost = sm([B, 16], F32, f"o_{wnm}")
        nc.scalar.copy(out=ost[:, 0:ncols], in_=ps[:, 0:ncols])
        nc.sync.dma_start(out=eout[onm], in_=ost[:, 0:ncols])


def kernel(**inputs):
    global LAST_RESULTS
    if "nc" not in _CACHE:
        _CACHE["nc"] = _build()
    nc = _CACHE["nc"]

    rep = {k: np.ascontiguousarray(np.asarray(inputs[k]), dtype=np.float32)
           for k in ("W1", "b1", "ln1_g", "ln1_b", "W2", "b2", "ln2_g",
                     "ln2_b", "codebook", "dW1", "db1", "dW2", "db2",
                     "pW", "pb", "prW", "prb", "vW", "vb")}
    u_full = {m: np.asarray(inputs[f"u_{m}"], dtype=np.float32)
              for m in ("t", "a", "v")}

    in_maps = []
    for c in range(N_CORES):
        sl = slice(c * B, (c + 1) * B)
        mp = dict(rep)
        for m in ("t", "a", "v"):
            mp[f"u_{m}"] = np.ascontiguousarray(u_full[m][sl])
        in_maps.append(mp)

    global _LAST_IN_MAPS
    _LAST_IN_MAPS = in_maps
    res = run_bass_kernel_spmd(nc, in_maps, list(range(N_CORES)))
    LAST_RESULTS = res
    rs = res.results

    cat = lambda nm: np.concatenate([rs[c][nm] for c in range(N_CORES)], 0)
    M = cat("M_out")
    m_raw = cat("mraw_out")
    spk = cat("spk_out")
    pol = cat("pol_out")
    pros = cat("pros_out")
    vis = cat("vis_out")
    vq_loss = np.float32(rs[0]["scal_out"][0, 0])
    perplexity = np.float32(rs[0]["scal_out"][0, 1])
    return (M, m_raw, vq_loss, perplexity, spk, pol, pros, vis)
